# revision 6
# baseline (speedup 1.0000x reference)
"""PhysicsGNN node-classification kernel for 8 TRN2 NeuronCores.

Strategy: shard nodes (and their incident edges, CSR-sorted by destination)
across 8 cores. Each layer: every core computes out_p = h @ W_eff.T for its
node shard, an AllGather builds the full out_p table in HBM, then each core
gathers source rows for its edges via dma_gather, builds coef-scaled one-hot
matrices on the DVE (iota == col_local) * coef, and scatter-adds contributions
into PSUM with the tensor engine (contraction over 128-edge chunks).
The -h*ext_w and +beta*x0 terms are injected into the same PSUM accumulation
as two extra matmuls; relu(0.1*conv) runs on ACT and h += on DVE.
"""

import numpy as np

# Problem constants (hardcoded per contract)
N_NODES = 50000
N_EDGES = 1600000
F_IN = 500
H = 64
C_OUT = 40
N_LAYERS = 4
STEP = 0.1
N_CORES = 8


def _pairwise_weight_np(pw_raw):
    W0 = np.triu(pw_raw[:, :-2], 1)
    W0 = W0 + W0.T
    q = pw_raw[:, -2]
    r = pw_raw[:, -1]
    w_diag = q * np.abs(W0).sum(axis=1) + r
    return (W0 + np.diag(w_diag)).astype(np.float32)


def _prep(x, edge_index, enc_w, dec_w, ext_w, beta, pw_raw, n_cores=N_CORES):
    """Host-side preprocessing: graph partitioning + CSR layout per core.

    Returns (in_maps, cfg) where cfg holds the compile-time sizes.
    """
    n_nodes = x.shape[0]
    f_in = x.shape[1]
    h = enc_w.shape[0]
    assert h == 64
    npc = n_nodes // n_cores            # nodes per core
    nblk = (npc + 127) // 128           # 128-node dest blocks per core
    npad = nblk * 128                   # padded nodes per core
    lo_rows = (n_cores // 2) * npad     # first half of the gather table

    row = edge_index[0].astype(np.int64)
    col = edge_index[1].astype(np.int64)

    deg = np.bincount(col, minlength=n_nodes).astype(np.float32)
    deg_inv = np.where(deg > 0, deg.astype(np.float32) ** -0.5, 0.0).astype(np.float32)
    coef = (deg_inv[row] * deg_inv[col]).astype(np.float32)

    # global node id -> padded table row
    trow_all = (row // npc) * npad + (row % npc)

    # Per-core edge layout. First pass: compute chunk counts to fix KL/KH.
    per_core = []
    for c in range(n_cores):
        m = (col // npc) == c
        e_cl = (col[m] - c * npc).astype(np.int64)   # col local 0..npc-1
        e_tr = trow_all[m]
        e_cf = coef[m]
        blk = e_cl // 128
        islow = e_tr < lo_rows
        # sort: block, low-before-high, then ascending source row (HBM locality)
        order = np.lexsort((e_tr, (~islow).astype(np.int8), blk))
        per_core.append((e_cl[order], e_tr[order], e_cf[order], blk[order],
                         islow[order]))

    KL = 1
    KH = 1
    for (e_cl, e_tr, e_cf, blk, islow) in per_core:
        for b in range(nblk):
            sel = blk == b
            nlo = int(np.count_nonzero(sel & islow))
            nhi = int(np.count_nonzero(sel & ~islow))
            KL = max(KL, (nlo + 127) // 128)
            KH = max(KH, (nhi + 127) // 128)
    KT = KL + KH

    # Shared small tensors
    W_eff = _pairwise_weight_np(pw_raw.astype(np.float32))
    w_rhs = np.ascontiguousarray(W_eff.T).astype(np.float32)         # [64,64]
    dec_rhs = np.ascontiguousarray(dec_w.astype(np.float32).T)       # [64,40]
    f_pad = ((f_in + 127) // 128) * 128
    enc_wT = np.zeros((f_pad, h), np.float32)
    enc_wT[:f_in] = enc_w.astype(np.float32).T
    eye64 = np.eye(h, dtype=np.float32)
    negext = (-np.diag(ext_w[0].astype(np.float32))).astype(np.float32)
    iota_t = np.broadcast_to(np.arange(128, dtype=np.float32), (128, 128)).copy()
    beta_val = float(np.asarray(beta).reshape(-1)[0])

    in_maps = []
    for c in range(n_cores):
        e_cl, e_tr, e_cf, blk, islow = per_core[c]

        gidx_lo = np.zeros((16, nblk * 8 * KL), np.int16)
        gidx_hi = np.zeros((16, nblk * 8 * KH), np.int16)
        colloc = np.zeros((128, nblk * KT), np.float32)
        coefar = np.zeros((128, nblk * KT), np.float32)

        for b in range(nblk):
            sel = blk == b
            lo_m = sel & islow
            hi_m = sel & ~islow
            for (msk, K0, koff, gidx, base) in (
                (lo_m, KL, 0, gidx_lo, 0),
                (hi_m, KH, KL, gidx_hi, lo_rows),
            ):
                tr = e_tr[msk] - base
                cl = e_cl[msk] % 128
                cf = e_cf[msk]
                n = tr.shape[0]
                # gather order position i -> lane i%128, chunk i//128
                idx_pad = np.zeros(128 * K0, np.int64)
                idx_pad[:n] = tr
                # index sbuf layout: position i -> [i%16, i//16]
                gidx[:, b * 8 * K0:(b + 1) * 8 * K0] = (
                    idx_pad.reshape(8 * K0, 16).T.astype(np.int16))
                lane = np.arange(n) % 128
                cc = np.arange(n) // 128
                colloc[lane, b * KT + koff + cc] = cl.astype(np.float32)
                coefar[lane, b * KT + koff + cc] = cf

        xT = np.zeros((f_pad, npad), np.float32)
        xT[:f_in, :npc] = x[c * npc:(c + 1) * npc].astype(np.float32).T

        in_maps.append({
            "xT": xT,
            "gidx_lo": np.ascontiguousarray(np.tile(gidx_lo, (8, 1))),
            "gidx_hi": np.ascontiguousarray(np.tile(gidx_hi, (8, 1))),
            "colloc": colloc,
            "coef": coefar,
            "enc_wT": enc_wT,
            "w_rhs": w_rhs,
            "dec_rhs": dec_rhs,
            "eye64": eye64,
            "negext": negext,
            "iota": iota_t,
        })

    cfg = dict(n_cores=n_cores, npc=npc, npad=npad, nblk=nblk, f_pad=f_pad,
               KL=KL, KH=KH, KT=KT, lo_rows=lo_rows, beta=beta_val, h=h,
               c_out=dec_w.shape[0], n_layers=N_LAYERS, step=STEP)
    return in_maps, cfg


def _build(cfg):
    import concourse.bacc as bacc
    import concourse.bass as bass
    import concourse.tile as tile
    import concourse.mybir as mybir

    f32 = mybir.dt.float32
    i16 = mybir.dt.int16
    AF = mybir.ActivationFunctionType
    OP = mybir.AluOpType

    n_cores = cfg["n_cores"]
    npad = cfg["npad"]
    nblk = cfg["nblk"]
    f_pad = cfg["f_pad"]
    KL, KH, KT = cfg["KL"], cfg["KH"], cfg["KT"]
    lo_rows = cfg["lo_rows"]
    h = cfg["h"]
    c_out = cfg["c_out"]
    kf = f_pad // 128

    nc = bacc.Bacc("TRN2", target_bir_lowering=False, debug=False,
                   num_devices=n_cores)

    xT_d = nc.dram_tensor("xT", [f_pad, npad], f32, kind="ExternalInput")
    gl_d = nc.dram_tensor("gidx_lo", [128, nblk * 8 * KL], i16, kind="ExternalInput")
    gh_d = nc.dram_tensor("gidx_hi", [128, nblk * 8 * KH], i16, kind="ExternalInput")
    cl_d = nc.dram_tensor("colloc", [128, nblk * KT], f32, kind="ExternalInput")
    cf_d = nc.dram_tensor("coef", [128, nblk * KT], f32, kind="ExternalInput")
    ew_d = nc.dram_tensor("enc_wT", [f_pad, h], f32, kind="ExternalInput")
    wr_d = nc.dram_tensor("w_rhs", [h, h], f32, kind="ExternalInput")
    dr_d = nc.dram_tensor("dec_rhs", [h, c_out], f32, kind="ExternalInput")
    ey_d = nc.dram_tensor("eye64", [h, h], f32, kind="ExternalInput")
    ne_d = nc.dram_tensor("negext", [h, h], f32, kind="ExternalInput")
    io_d = nc.dram_tensor("iota", [128, 128], f32, kind="ExternalInput")
    out_d = nc.dram_tensor("out", [npad, c_out], f32, kind="ExternalOutput")

    rg = [list(range(n_cores))]

    with tile.TileContext(nc, num_cores=n_cores) as tc:
        with (
            tc.tile_pool(name="const", bufs=1) as constp,
            tc.tile_pool(name="dram", bufs=1, space="DRAM") as dramp,
            tc.tile_pool(name="gather", bufs=2) as gpool,
            tc.tile_pool(name="onehot", bufs=4) as ohpool,
            tc.tile_pool(name="relu", bufs=2) as rpool,
            tc.tile_pool(name="agg_ps", bufs=2, space="PSUM") as aggps,
            tc.tile_pool(name="op_ps", bufs=2, space="PSUM") as opps,
        ):
            cc_in = dramp.tile([npad, h], f32)
            tables = [
                dramp.tile([n_cores * npad, h], f32, addr_space="Shared",
                           name=f"table{i}")
                for i in range(cfg["n_layers"])
            ]

            hT = constp.tile([h, npad], f32)
            zT = constp.tile([h, npad], f32)
            op_stage = constp.tile([128, nblk * h], f32)
            out_stage = constp.tile([128, nblk * c_out], f32)

            # small constants
            enc_sb = constp.tile([128, kf * h], f32)
            nc.sync.dma_start(
                out=enc_sb[:].rearrange("p (k h) -> p k h", k=kf),
                in_=ew_d.ap().rearrange("(k p) h -> p k h", p=128))
            wr_sb = constp.tile([h, h], f32)
            nc.sync.dma_start(out=wr_sb[:], in_=wr_d[:, :])
            dr_sb = constp.tile([h, c_out], f32)
            nc.sync.dma_start(out=dr_sb[:], in_=dr_d[:, :])
            ey_sb = constp.tile([h, h], f32)
            nc.sync.dma_start(out=ey_sb[:], in_=ey_d[:, :])
            ne_sb = constp.tile([h, h], f32)
            nc.sync.dma_start(out=ne_sb[:], in_=ne_d[:, :])
            io_sb = constp.tile([128, 128], f32)
            nc.sync.dma_start(out=io_sb[:], in_=io_d[:, :])
            cl_sb = constp.tile([128, nblk * KT], f32)
            nc.sync.dma_start(out=cl_sb[:], in_=cl_d[:, :])
            cf_sb = constp.tile([128, nblk * KT], f32)
            nc.sync.dma_start(out=cf_sb[:], in_=cf_d[:, :])
            gl_sb = constp.tile([128, nblk * 8 * KL], i16)
            nc.sync.dma_start(out=gl_sb[:], in_=gl_d[:, :])
            gh_sb = constp.tile([128, nblk * 8 * KH], i16)
            nc.sync.dma_start(out=gh_sb[:], in_=gh_d[:, :])

            # ---------------- encoder ----------------
            with (
                tc.tile_pool(name="xtile", bufs=3) as xpool,
                tc.tile_pool(name="enc_ps", bufs=2, space="PSUM") as encps,
            ):
                ntile = 512
                nt_enc = (npad + ntile - 1) // ntile
                for t in range(nt_enc):
                    n0 = t * ntile
                    nt = min(ntile, npad - n0)
                    xt = xpool.tile([128, kf, ntile], f32)
                    nc.sync.dma_start(
                        out=xt[:, :, :nt],
                        in_=xT_d.ap().rearrange("(k p) n -> p k n", p=128)
                            [:, :, n0:n0 + nt])
                    ps = encps.tile([h, ntile], f32)
                    for k in range(kf):
                        nc.tensor.matmul(ps[:, :nt],
                                         lhsT=enc_sb[:, k * h:(k + 1) * h],
                                         rhs=xt[:, k, :nt],
                                         start=(k == 0), stop=(k == kf - 1))
                    nc.vector.tensor_copy(hT[:, n0:n0 + nt], ps[:, :nt])
                    nc.scalar.activation(zT[:, n0:n0 + nt], ps[:, :nt],
                                         AF.Copy, scale=cfg["beta"])

            # ---------------- layers ----------------
            for layer in range(cfg["n_layers"]):
                table = tables[layer]
                # out_p shard for this layer
                for b in range(nblk):
                    ps = opps.tile([128, h], f32)
                    nc.tensor.matmul(ps[:, :], lhsT=hT[:, b * 128:(b + 1) * 128],
                                     rhs=wr_sb[:, :], start=True, stop=True)
                    nc.vector.tensor_copy(op_stage[:, b * h:(b + 1) * h], ps[:, :])
                nc.sync.dma_start(
                    out=cc_in[:].rearrange("(b p) h -> p b h", p=128),
                    in_=op_stage[:].rearrange("p (b h) -> p b h", b=nblk))
                nc.gpsimd.collective_compute(
                    "AllGather", mybir.AluOpType.bypass, replica_groups=rg,
                    ins=[cc_in[:].opt()], outs=[table[:].opt()])

                for b in range(nblk):
                    g = gpool.tile([128, KT, h], f32)
                    nc.gpsimd.dma_gather(
                        g[:, 0:KL, :], table[0:lo_rows, :],
                        gl_sb[:, b * 8 * KL:(b + 1) * 8 * KL],
                        128 * KL, 128 * KL, h)
                    nc.gpsimd.dma_gather(
                        g[:, KL:KT, :], table[lo_rows:2 * lo_rows, :],
                        gh_sb[:, b * 8 * KH:(b + 1) * 8 * KH],
                        128 * KH, 128 * KH, h)
                    ps = aggps.tile([h, 128], f32)
                    nc.tensor.matmul(ps[:, :], lhsT=ey_sb[:, :],
                                     rhs=zT[:, b * 128:(b + 1) * 128],
                                     start=True, stop=False)
                    nc.tensor.matmul(ps[:, :], lhsT=ne_sb[:, :],
                                     rhs=hT[:, b * 128:(b + 1) * 128],
                                     start=False, stop=False)
                    for cc in range(KT):
                        oh = ohpool.tile([128, 128], f32)
                        j = b * KT + cc
                        nc.vector.tensor_scalar(
                            oh[:, :], io_sb[:, :],
                            cl_sb[:, j:j + 1], cf_sb[:, j:j + 1],
                            op0=OP.is_equal, op1=OP.mult)
                        nc.tensor.matmul(ps[:, :], lhsT=g[:, cc, :], rhs=oh[:, :],
                                         start=False, stop=(cc == KT - 1))
                    r = rpool.tile([h, 128], f32)
                    nc.scalar.activation(r[:, :], ps[:, :], AF.Relu,
                                         scale=cfg["step"])
                    nc.vector.tensor_add(hT[:, b * 128:(b + 1) * 128],
                                         hT[:, b * 128:(b + 1) * 128], r[:, :])

            # ---------------- decoder ----------------
            for b in range(nblk):
                ps = opps.tile([128, c_out], f32, tag="dec_ps")
                nc.tensor.matmul(ps[:, :], lhsT=hT[:, b * 128:(b + 1) * 128],
                                 rhs=dr_sb[:, :], start=True, stop=True)
                nc.vector.tensor_copy(out_stage[:, b * c_out:(b + 1) * c_out],
                                      ps[:, :])
            nc.sync.dma_start(
                out=out_d.ap().rearrange("(b p) c -> p b c", p=128),
                in_=out_stage[:].rearrange("p (b c) -> p b c", b=nblk))

    nc.compile()
    return nc


def _make_exec(nc, in_maps):
    """Build a reusable warm-executable mirror of bass2jax.run_bass_via_pjrt.

    Returns (run_once, out_names, out_avals) where run_once() executes the
    NEFF on all cores with device-resident inputs and returns per-core output
    dicts. Repeated calls reuse the compiled executable and input buffers.
    """
    import jax
    import jax.numpy as jnp
    from jax.sharding import Mesh, PartitionSpec
    from jax.experimental.shard_map import shard_map
    import concourse.mybir as mybir
    from concourse import bass2jax

    bass2jax.install_neuronx_cc_hook()
    n_cores = len(in_maps)

    partition_name = (nc.partition_id_tensor.name
                      if nc.partition_id_tensor else None)

    in_names, out_names, out_avals, zero_outs = [], [], [], []
    for alloc in nc.m.functions[0].allocations:
        if not isinstance(alloc, mybir.MemoryLocationSet):
            continue
        name = alloc.memorylocations[0].name
        if alloc.kind == "ExternalInput":
            if name != partition_name:
                in_names.append(name)
        elif alloc.kind == "ExternalOutput":
            out_names.append(name)
            shape = tuple(alloc.tensor_shape)
            dtype = mybir.dt.np(alloc.dtype)
            out_avals.append(jax.core.ShapedArray(shape, dtype))
            zero_outs.append(np.zeros(shape, dtype))
    n_params = len(in_names)
    n_outs = len(out_avals)
    all_names = in_names + out_names
    if partition_name is not None:
        all_names = all_names + [partition_name]

    def _body(*args):
        operands = list(args)
        if partition_name is not None:
            operands.append(bass2jax.partition_id_tensor())
        outs = bass2jax._bass_exec_p.bind(
            *operands,
            out_avals=tuple(out_avals),
            in_names=tuple(all_names),
            out_names=tuple(out_names),
            lowering_input_output_aliases=(),
            sim_require_finite=True,
            sim_require_nnan=True,
            nc=nc,
        )
        return tuple(outs)

    donate = tuple(range(n_params, n_params + n_outs))
    devices = jax.devices()[:n_cores]
    mesh = Mesh(np.asarray(devices), ("core",))
    sharded = jax.jit(
        shard_map(_body, mesh=mesh,
                  in_specs=(PartitionSpec("core"),) * (n_params + n_outs),
                  out_specs=(PartitionSpec("core"),) * n_outs,
                  check_rep=False),
        donate_argnums=donate, keep_unused=True)

    concat_in = [
        np.concatenate([np.asarray(in_maps[c][nm]) for c in range(n_cores)], axis=0)
        for nm in in_names
    ]
    concat_in_dev = jax.device_put(concat_in)
    jax.block_until_ready(concat_in_dev)
    concat_zero_np = [
        np.zeros((n_cores * z.shape[0], *z.shape[1:]), z.dtype) for z in zero_outs
    ]

    def run_once():
        zeros_dev = jax.device_put(concat_zero_np)
        jax.block_until_ready(zeros_dev)
        import time as _t
        t0 = _t.perf_counter()
        out_arrs = sharded(*concat_in_dev, *zeros_dev)
        jax.block_until_ready(out_arrs)
        dt = _t.perf_counter() - t0
        results = [
            {nm: np.asarray(out_arrs[i]).reshape(n_cores, *out_avals[i].shape)[c]
             for i, nm in enumerate(out_names)}
            for c in range(n_cores)
        ]
        return results, dt

    return run_once


def kernel(x, edge_index, enc_w, dec_w, ext_w, beta, pw_raw,
           _return_exec=False):
    in_maps, cfg = _prep(x, edge_index, enc_w, dec_w, ext_w, beta, pw_raw)
    nc = _build(cfg)
    run_once = _make_exec(nc, in_maps)
    results, dt = run_once()
    npc = cfg["npc"]
    out = np.concatenate([r["out"][:npc] for r in results], axis=0)
    out = out.astype(np.float32)
    if _return_exec:
        return out, run_once, dt
    return out


# revision 7
# speedup vs baseline: 272.9563x; 272.9563x over previous
"""PhysicsGNN node-classification kernel for 8 TRN2 NeuronCores.

Strategy: shard nodes (and their incident edges, CSR-sorted by destination)
across 8 cores. Each layer: every core computes out_p = h @ W_eff.T for its
node shard, an AllGather builds the full out_p table in HBM, then each core
gathers source rows for its edges via dma_gather, builds coef-scaled one-hot
matrices on the DVE (iota == col_local) * coef, and scatter-adds contributions
into PSUM with the tensor engine (contraction over 128-edge chunks).
The -h*ext_w and +beta*x0 terms are injected into the same PSUM accumulation
as two extra matmuls; relu(0.1*conv) runs on ACT and h += on DVE.
"""

import numpy as np

# Problem constants (hardcoded per contract)
N_NODES = 50000
N_EDGES = 1600000
F_IN = 500
H = 64
C_OUT = 40
N_LAYERS = 4
STEP = 0.1
N_CORES = 8


def _pairwise_weight_np(pw_raw):
    W0 = np.triu(pw_raw[:, :-2], 1)
    W0 = W0 + W0.T
    q = pw_raw[:, -2]
    r = pw_raw[:, -1]
    w_diag = q * np.abs(W0).sum(axis=1) + r
    return (W0 + np.diag(w_diag)).astype(np.float32)


def _prep(x, edge_index, enc_w, dec_w, ext_w, beta, pw_raw, n_cores=N_CORES):
    """Host-side preprocessing: graph partitioning + CSR layout per core.

    Returns (in_maps, cfg) where cfg holds the compile-time sizes.
    """
    n_nodes = x.shape[0]
    f_in = x.shape[1]
    h = enc_w.shape[0]
    assert h == 64
    npc = n_nodes // n_cores            # nodes per core
    nblk = (npc + 127) // 128           # 128-node dest blocks per core
    npad = nblk * 128                   # padded nodes per core
    lo_rows = (n_cores // 2) * npad     # first half of the gather table

    row = edge_index[0].astype(np.int64)
    col = edge_index[1].astype(np.int64)

    deg = np.bincount(col, minlength=n_nodes).astype(np.float32)
    deg_inv = np.where(deg > 0, deg.astype(np.float32) ** -0.5, 0.0).astype(np.float32)
    coef = (deg_inv[row] * deg_inv[col]).astype(np.float32)

    # global node id -> padded table row
    trow_all = (row // npc) * npad + (row % npc)

    # Per-core edge layout. First pass: compute chunk counts to fix KL/KH.
    per_core = []
    for c in range(n_cores):
        m = (col // npc) == c
        e_cl = (col[m] - c * npc).astype(np.int64)   # col local 0..npc-1
        e_tr = trow_all[m]
        e_cf = coef[m]
        blk = e_cl // 128
        islow = e_tr < lo_rows
        # sort: block, low-before-high, then ascending source row (HBM locality)
        order = np.lexsort((e_tr, (~islow).astype(np.int8), blk))
        per_core.append((e_cl[order], e_tr[order], e_cf[order], blk[order],
                         islow[order]))

    KL = 1
    KH = 1
    for (e_cl, e_tr, e_cf, blk, islow) in per_core:
        for b in range(nblk):
            sel = blk == b
            nlo = int(np.count_nonzero(sel & islow))
            nhi = int(np.count_nonzero(sel & ~islow))
            KL = max(KL, (nlo + 127) // 128)
            KH = max(KH, (nhi + 127) // 128)
    KT = KL + KH

    # Shared small tensors
    W_eff = _pairwise_weight_np(pw_raw.astype(np.float32))
    w_rhs = np.ascontiguousarray(W_eff.T).astype(np.float32)         # [64,64]
    dec_rhs = np.ascontiguousarray(dec_w.astype(np.float32).T)       # [64,40]
    f_pad = ((f_in + 127) // 128) * 128
    enc_wT = np.zeros((f_pad, h), np.float32)
    enc_wT[:f_in] = enc_w.astype(np.float32).T
    eye64 = np.eye(h, dtype=np.float32)
    negext = (-np.diag(ext_w[0].astype(np.float32))).astype(np.float32)
    iota_t = np.broadcast_to(np.arange(128, dtype=np.float32), (128, 128)).copy()
    beta_val = float(np.asarray(beta).reshape(-1)[0])

    in_maps = []
    for c in range(n_cores):
        e_cl, e_tr, e_cf, blk, islow = per_core[c]

        gidx_lo = np.zeros((16, nblk * 8 * KL), np.int16)
        gidx_hi = np.zeros((16, nblk * 8 * KH), np.int16)
        colloc = np.zeros((128, nblk * KT), np.float32)
        coefar = np.zeros((128, nblk * KT), np.float32)

        for b in range(nblk):
            sel = blk == b
            lo_m = sel & islow
            hi_m = sel & ~islow
            for (msk, K0, koff, gidx, base) in (
                (lo_m, KL, 0, gidx_lo, 0),
                (hi_m, KH, KL, gidx_hi, lo_rows),
            ):
                tr = e_tr[msk] - base
                cl = e_cl[msk] % 128
                cf = e_cf[msk]
                n = tr.shape[0]
                # gather order position i -> lane i%128, chunk i//128
                idx_pad = np.zeros(128 * K0, np.int64)
                idx_pad[:n] = tr
                # index sbuf layout: position i -> [i%16, i//16]
                gidx[:, b * 8 * K0:(b + 1) * 8 * K0] = (
                    idx_pad.reshape(8 * K0, 16).T.astype(np.int16))
                lane = np.arange(n) % 128
                cc = np.arange(n) // 128
                colloc[lane, b * KT + koff + cc] = cl.astype(np.float32)
                coefar[lane, b * KT + koff + cc] = cf

        xT = np.zeros((f_pad, npad), np.float32)
        xT[:f_in, :npc] = x[c * npc:(c + 1) * npc].astype(np.float32).T

        in_maps.append({
            "xT": xT,
            "gidx_lo": np.ascontiguousarray(np.tile(gidx_lo, (8, 1))),
            "gidx_hi": np.ascontiguousarray(np.tile(gidx_hi, (8, 1))),
            "colloc": colloc,
            "coef": coefar,
            "enc_wT": enc_wT,
            "w_rhs": w_rhs,
            "dec_rhs": dec_rhs,
            "eye64": eye64,
            "negext": negext,
            "iota": iota_t,
        })

    cfg = dict(n_cores=n_cores, npc=npc, npad=npad, nblk=nblk, f_pad=f_pad,
               KL=KL, KH=KH, KT=KT, lo_rows=lo_rows, beta=beta_val, h=h,
               c_out=dec_w.shape[0], n_layers=N_LAYERS, step=STEP)
    return in_maps, cfg


def _build(cfg):
    import concourse.bacc as bacc
    import concourse.bass as bass
    import concourse.tile as tile
    import concourse.mybir as mybir

    f32 = mybir.dt.float32
    i16 = mybir.dt.int16
    AF = mybir.ActivationFunctionType
    OP = mybir.AluOpType

    n_cores = cfg["n_cores"]
    npad = cfg["npad"]
    nblk = cfg["nblk"]
    f_pad = cfg["f_pad"]
    KL, KH, KT = cfg["KL"], cfg["KH"], cfg["KT"]
    lo_rows = cfg["lo_rows"]
    h = cfg["h"]
    c_out = cfg["c_out"]
    kf = f_pad // 128

    nc = bacc.Bacc("TRN2", target_bir_lowering=False, debug=False,
                   num_devices=n_cores)

    xT_d = nc.dram_tensor("xT", [f_pad, npad], f32, kind="ExternalInput")
    gl_d = nc.dram_tensor("gidx_lo", [128, nblk * 8 * KL], i16, kind="ExternalInput")
    gh_d = nc.dram_tensor("gidx_hi", [128, nblk * 8 * KH], i16, kind="ExternalInput")
    cl_d = nc.dram_tensor("colloc", [128, nblk * KT], f32, kind="ExternalInput")
    cf_d = nc.dram_tensor("coef", [128, nblk * KT], f32, kind="ExternalInput")
    ew_d = nc.dram_tensor("enc_wT", [f_pad, h], f32, kind="ExternalInput")
    wr_d = nc.dram_tensor("w_rhs", [h, h], f32, kind="ExternalInput")
    dr_d = nc.dram_tensor("dec_rhs", [h, c_out], f32, kind="ExternalInput")
    ey_d = nc.dram_tensor("eye64", [h, h], f32, kind="ExternalInput")
    ne_d = nc.dram_tensor("negext", [h, h], f32, kind="ExternalInput")
    io_d = nc.dram_tensor("iota", [128, 128], f32, kind="ExternalInput")
    out_d = nc.dram_tensor("out", [npad, c_out], f32, kind="ExternalOutput")

    rg = [list(range(n_cores))]

    with tile.TileContext(nc, num_cores=n_cores) as tc:
        with (
            tc.tile_pool(name="const", bufs=1) as constp,
            tc.tile_pool(name="dram", bufs=1, space="DRAM") as dramp,
            tc.tile_pool(name="gather", bufs=2) as gpool,
            tc.tile_pool(name="onehot", bufs=4) as ohpool,
            tc.tile_pool(name="relu", bufs=2) as rpool,
            tc.tile_pool(name="agg_ps", bufs=2, space="PSUM") as aggps,
            tc.tile_pool(name="op_ps", bufs=2, space="PSUM") as opps,
        ):
            cc_in = dramp.tile([npad, h], f32)
            tables = [
                dramp.tile([n_cores * npad, h], f32, addr_space="Shared",
                           name=f"table{i}")
                for i in range(cfg["n_layers"])
            ]

            hT = constp.tile([h, npad], f32)
            zT = constp.tile([h, npad], f32)
            op_stage = constp.tile([128, nblk * h], f32)
            out_stage = constp.tile([128, nblk * c_out], f32)

            # small constants
            enc_sb = constp.tile([128, kf * h], f32)
            nc.sync.dma_start(
                out=enc_sb[:].rearrange("p (k h) -> p k h", k=kf),
                in_=ew_d.ap().rearrange("(k p) h -> p k h", p=128))
            wr_sb = constp.tile([h, h], f32)
            nc.sync.dma_start(out=wr_sb[:], in_=wr_d[:, :])
            dr_sb = constp.tile([h, c_out], f32)
            nc.sync.dma_start(out=dr_sb[:], in_=dr_d[:, :])
            ey_sb = constp.tile([h, h], f32)
            nc.sync.dma_start(out=ey_sb[:], in_=ey_d[:, :])
            ne_sb = constp.tile([h, h], f32)
            nc.sync.dma_start(out=ne_sb[:], in_=ne_d[:, :])
            io_sb = constp.tile([128, 128], f32)
            nc.sync.dma_start(out=io_sb[:], in_=io_d[:, :])
            cl_sb = constp.tile([128, nblk * KT], f32)
            nc.sync.dma_start(out=cl_sb[:], in_=cl_d[:, :])
            cf_sb = constp.tile([128, nblk * KT], f32)
            nc.sync.dma_start(out=cf_sb[:], in_=cf_d[:, :])
            gl_sb = constp.tile([128, nblk * 8 * KL], i16)
            nc.sync.dma_start(out=gl_sb[:], in_=gl_d[:, :])
            gh_sb = constp.tile([128, nblk * 8 * KH], i16)
            nc.sync.dma_start(out=gh_sb[:], in_=gh_d[:, :])

            # ---------------- encoder ----------------
            with (
                tc.tile_pool(name="xtile", bufs=3) as xpool,
                tc.tile_pool(name="enc_ps", bufs=2, space="PSUM") as encps,
            ):
                ntile = 512
                nt_enc = (npad + ntile - 1) // ntile
                for t in range(nt_enc):
                    n0 = t * ntile
                    nt = min(ntile, npad - n0)
                    xt = xpool.tile([128, kf, ntile], f32)
                    nc.sync.dma_start(
                        out=xt[:, :, :nt],
                        in_=xT_d.ap().rearrange("(k p) n -> p k n", p=128)
                            [:, :, n0:n0 + nt])
                    ps = encps.tile([h, ntile], f32)
                    for k in range(kf):
                        nc.tensor.matmul(ps[:, :nt],
                                         lhsT=enc_sb[:, k * h:(k + 1) * h],
                                         rhs=xt[:, k, :nt],
                                         start=(k == 0), stop=(k == kf - 1))
                    nc.vector.tensor_copy(hT[:, n0:n0 + nt], ps[:, :nt])
                    nc.scalar.activation(zT[:, n0:n0 + nt], ps[:, :nt],
                                         AF.Copy, scale=cfg["beta"])

            # ---------------- layers ----------------
            for layer in range(cfg["n_layers"]):
                table = tables[layer]
                # out_p shard for this layer
                for b in range(nblk):
                    ps = opps.tile([128, h], f32)
                    nc.tensor.matmul(ps[:, :], lhsT=hT[:, b * 128:(b + 1) * 128],
                                     rhs=wr_sb[:, :], start=True, stop=True)
                    nc.vector.tensor_copy(op_stage[:, b * h:(b + 1) * h], ps[:, :])
                nc.sync.dma_start(
                    out=cc_in[:].rearrange("(b p) h -> p b h", p=128),
                    in_=op_stage[:].rearrange("p (b h) -> p b h", b=nblk))
                nc.gpsimd.collective_compute(
                    "AllGather", mybir.AluOpType.bypass, replica_groups=rg,
                    ins=[cc_in[:].opt()], outs=[table[:].opt()])

                for b in range(nblk):
                    g = gpool.tile([128, KT, h], f32)
                    nc.gpsimd.dma_gather(
                        g[:, 0:KL, :], table[0:lo_rows, :],
                        gl_sb[:, b * 8 * KL:(b + 1) * 8 * KL],
                        128 * KL, 128 * KL, h)
                    nc.gpsimd.dma_gather(
                        g[:, KL:KT, :], table[lo_rows:2 * lo_rows, :],
                        gh_sb[:, b * 8 * KH:(b + 1) * 8 * KH],
                        128 * KH, 128 * KH, h)
                    ps = aggps.tile([h, 128], f32)
                    nc.tensor.matmul(ps[:, :], lhsT=ey_sb[:, :],
                                     rhs=zT[:, b * 128:(b + 1) * 128],
                                     start=True, stop=False)
                    nc.tensor.matmul(ps[:, :], lhsT=ne_sb[:, :],
                                     rhs=hT[:, b * 128:(b + 1) * 128],
                                     start=False, stop=False)
                    for cc in range(KT):
                        oh = ohpool.tile([128, 128], f32)
                        j = b * KT + cc
                        nc.vector.tensor_scalar(
                            oh[:, :], io_sb[:, :],
                            cl_sb[:, j:j + 1], cf_sb[:, j:j + 1],
                            op0=OP.is_equal, op1=OP.mult)
                        nc.tensor.matmul(ps[:, :], lhsT=g[:, cc, :], rhs=oh[:, :],
                                         start=False, stop=(cc == KT - 1))
                    r = rpool.tile([h, 128], f32)
                    nc.scalar.activation(r[:, :], ps[:, :], AF.Relu,
                                         scale=cfg["step"])
                    nc.vector.tensor_add(hT[:, b * 128:(b + 1) * 128],
                                         hT[:, b * 128:(b + 1) * 128], r[:, :])

            # ---------------- decoder ----------------
            for b in range(nblk):
                ps = opps.tile([128, c_out], f32, tag="dec_ps")
                nc.tensor.matmul(ps[:, :], lhsT=hT[:, b * 128:(b + 1) * 128],
                                 rhs=dr_sb[:, :], start=True, stop=True)
                nc.vector.tensor_copy(out_stage[:, b * c_out:(b + 1) * c_out],
                                      ps[:, :])
            nc.sync.dma_start(
                out=out_d.ap().rearrange("(b p) c -> p b c", p=128),
                in_=out_stage[:].rearrange("p (b c) -> p b c", b=nblk))

    nc.compile()
    return nc


def _make_exec(nc, in_maps):
    """Build a reusable warm-executable mirror of bass2jax.run_bass_via_pjrt.

    Returns (run_once, out_names, out_avals) where run_once() executes the
    NEFF on all cores with device-resident inputs and returns per-core output
    dicts. Repeated calls reuse the compiled executable and input buffers.
    """
    import jax
    import jax.numpy as jnp
    from jax.sharding import Mesh, PartitionSpec
    from jax.experimental.shard_map import shard_map
    import concourse.mybir as mybir
    from concourse import bass2jax

    bass2jax.install_neuronx_cc_hook()
    n_cores = len(in_maps)

    partition_name = (nc.partition_id_tensor.name
                      if nc.partition_id_tensor else None)

    in_names, out_names, out_avals, zero_outs = [], [], [], []
    for alloc in nc.m.functions[0].allocations:
        if not isinstance(alloc, mybir.MemoryLocationSet):
            continue
        name = alloc.memorylocations[0].name
        if alloc.kind == "ExternalInput":
            if name != partition_name:
                in_names.append(name)
        elif alloc.kind == "ExternalOutput":
            out_names.append(name)
            shape = tuple(alloc.tensor_shape)
            dtype = mybir.dt.np(alloc.dtype)
            out_avals.append(jax.core.ShapedArray(shape, dtype))
            zero_outs.append(np.zeros(shape, dtype))
    n_params = len(in_names)
    n_outs = len(out_avals)
    all_names = in_names + out_names
    if partition_name is not None:
        all_names = all_names + [partition_name]

    def _body(*args):
        operands = list(args)
        if partition_name is not None:
            operands.append(bass2jax.partition_id_tensor())
        outs = bass2jax._bass_exec_p.bind(
            *operands,
            out_avals=tuple(out_avals),
            in_names=tuple(all_names),
            out_names=tuple(out_names),
            lowering_input_output_aliases=(),
            sim_require_finite=True,
            sim_require_nnan=True,
            nc=nc,
        )
        return tuple(outs)

    donate = tuple(range(n_params, n_params + n_outs))
    devices = jax.devices()[:n_cores]
    mesh = Mesh(np.asarray(devices), ("core",))
    sharded = jax.jit(
        shard_map(_body, mesh=mesh,
                  in_specs=(PartitionSpec("core"),) * (n_params + n_outs),
                  out_specs=(PartitionSpec("core"),) * n_outs,
                  check_rep=False),
        donate_argnums=donate, keep_unused=True)

    concat_in = [
        np.concatenate([np.asarray(in_maps[c][nm]) for c in range(n_cores)], axis=0)
        for nm in in_names
    ]
    concat_in_dev = jax.device_put(concat_in)
    jax.block_until_ready(concat_in_dev)
    concat_zero_np = [
        np.zeros((n_cores * z.shape[0], *z.shape[1:]), z.dtype) for z in zero_outs
    ]

    def run_once():
        zeros_dev = jax.device_put(concat_zero_np)
        jax.block_until_ready(zeros_dev)
        import time as _t
        t0 = _t.perf_counter()
        out_arrs = sharded(*concat_in_dev, *zeros_dev)
        jax.block_until_ready(out_arrs)
        dt = _t.perf_counter() - t0
        results = [
            {nm: np.asarray(out_arrs[i]).reshape(n_cores, *out_avals[i].shape)[c]
             for i, nm in enumerate(out_names)}
            for c in range(n_cores)
        ]
        return results, dt

    return run_once


_LAST_NC = None


def kernel(x, edge_index, enc_w, dec_w, ext_w, beta, pw_raw,
           _return_exec=False):
    global _LAST_NC
    in_maps, cfg = _prep(x, edge_index, enc_w, dec_w, ext_w, beta, pw_raw)
    nc = _build(cfg)
    _LAST_NC = nc
    run_once = _make_exec(nc, in_maps)
    results, dt = run_once()
    npc = cfg["npc"]
    out = np.concatenate([r["out"][:npc] for r in results], axis=0)
    out = out.astype(np.float32)
    if _return_exec:
        return out, run_once, dt
    return out
